# revision 10
# baseline (speedup 1.0000x reference)
"""Trainium2 Bass kernel for nn_Predictor (segment-mean + embedding + fused linears).

Model (reference):
    mora_feat = segment_mean(features, mora_index)        # [B, M, D], sorted contiguous segments
    mv        = emb_table[vowels]                          # [B, M, VE]
    mh        = concat([mv, mora_feat]) @ W_mora + b_mora  # [B, M, H]
    (fh = features @ W_frame + b_frame is computed then deleted -> dead code, skipped)
    out       = mh @ W_post + b_post                       # [B, M, 8] -> [B, M, 2, 4]

Since there is no nonlinearity between the two linears they fold:
    out = concat([mv, mora_feat]) @ (W_mora @ W_post) + (b_mora @ W_post + b_post)
W_eff = W_mora @ W_post and b_eff are computed once on device.

Strategy (8 cores, data-parallel over batch: 2 utterances/core):
  - segment sums computed on TensorE as feat_chunk.T @ onehot(mora_index) with the
    output laid out transposed: sums^T [D, M].  Because mora_index is sorted, each
    256-frame superchunk only touches a narrow static window of mora columns; the
    window schedule is derived from the actual input at trace time (falls back to
    full width if the data violates it), so the single SPMD trace is correct for
    the given inputs by construction.
  - counts via a ones-column matmul into a [1, M] psum row; mean scaling is folded
    in as a per-column multiply (inv counts broadcast via a K=1 matmul).
  - vowel embedding gather as emb_table.T @ onehot(vowels) (V=50 is tiny).
  - final: out^T [8, M] = W_eff_aug.T @ concat_aug^T with bias via a ones row.
"""

import os
import sys

import numpy as np

B, F, M, D = 16, 4096, 512, 256
VE, H, V, OUT = 64, 512, 50, 8
N_CORES = 8
U = B // N_CORES          # utterances per core
SC = 16                   # superchunks per utterance (256 frames each)
FPS = F // SC             # frames per superchunk = 256
NQ = 4                    # feature DMAs per utterance (4 superchunks = 1 MiB each)

_TRACE = bool(os.environ.get("KERNEL_TRACE"))
LAST_EXEC_NS = None

_cache = {}


def _import_bass():
    for p in ("/opt/trn_rl_repo",):
        if p not in sys.path:
            sys.path.insert(0, p)
    import concourse.bass as bass
    import concourse.tile as tile
    from concourse import bacc, mybir
    return bass, tile, bacc, mybir


def _window_schedule(mora):
    """Static per-superchunk mora windows covering every utterance's data."""
    lo = np.full(SC, 0, np.int64)
    hi = np.full(SC, M - 1, np.int64)
    for s in range(SC):
        seg = mora[:, s * FPS:(s + 1) * FPS]
        lo[s] = int(seg.min())
        hi[s] = int(seg.max())
    w = int((hi - lo + 1).max())
    w = min(M, max(32, ((w + 15) // 16) * 16))
    starts = np.minimum(lo, M - w).astype(np.int64)
    # sanity: windows must cover the data (always true by construction)
    assert all(lo[s] >= starts[s] and hi[s] < starts[s] + w for s in range(SC))
    return int(w), tuple(int(x) for x in starts)


def _build_nc(win_w, starts):
    bass, tile, bacc, mybir = _import_bass()
    from contextlib import ExitStack
    f32 = mybir.dt.float32
    i32 = mybir.dt.int32
    ALU = mybir.AluOpType

    nc = bacc.Bacc()
    feat_in = nc.declare_dram_parameter("features", [U, F, D], f32, isOutput=False)
    morat_in = nc.declare_dram_parameter("morat", [U, 128, SC * 2], i32, isOutput=False)
    vow_in = nc.declare_dram_parameter("vowels", [U, M], i32, isOutput=False)
    emb_in = nc.declare_dram_parameter("emb", [V, VE], f32, isOutput=False)
    wmT_in = nc.declare_dram_parameter("W_moraT", [H, VE + D], f32, isOutput=False)
    wp_in = nc.declare_dram_parameter("W_post", [H, OUT], f32, isOutput=False)
    bm_in = nc.declare_dram_parameter("b_mora", [H], f32, isOutput=False)
    bp_in = nc.declare_dram_parameter("b_post", [1, OUT], f32, isOutput=False)
    out_dram = nc.declare_dram_parameter("out", [U, OUT, M], f32, isOutput=True)

    KA = VE + 1  # emb rows + ones row (bias)

    with tile.TileContext(nc) as tc:
        with ExitStack() as ctx:
            const = ctx.enter_context(tc.tile_pool(name="const", bufs=1))
            sb = ctx.enter_context(tc.tile_pool(name="sb", bufs=2))
            featp = ctx.enter_context(tc.tile_pool(name="featp", bufs=4))
            ohp = ctx.enter_context(tc.tile_pool(name="ohp", bufs=4))
            psA = ctx.enter_context(tc.tile_pool(name="psA", bufs=2, space="PSUM"))
            psB = ctx.enter_context(tc.tile_pool(name="psB", bufs=2, space="PSUM"))
            psC = ctx.enter_context(tc.tile_pool(name="psC", bufs=2, space="PSUM"))
            psX = ctx.enter_context(tc.tile_pool(name="psX", bufs=2, space="PSUM"))

            # ---- constants ----
            iota_i = const.tile([128, M], i32)
            nc.gpsimd.iota(iota_i[:], pattern=[[1, M]], base=0, channel_multiplier=0)
            iota_f = const.tile([128, M], f32)
            nc.vector.tensor_copy(iota_f[:], iota_i[:])
            iotac_i = const.tile([128, 1], i32)
            nc.gpsimd.iota(iotac_i[:], pattern=[[1, 1]], base=0, channel_multiplier=1)
            iotac_f = const.tile([128, 1], f32)
            nc.vector.tensor_copy(iotac_f[:], iotac_i[:])
            ones_col = const.tile([128, 1], f32)
            nc.vector.memset(ones_col[:], 1.0)
            ones_row = const.tile([1, 128], f32)
            nc.vector.memset(ones_row[:], 1.0)

            emb_sb = const.tile([V, VE], f32)
            nc.sync.dma_start(emb_sb[:], emb_in[:, :])
            wm_sb = const.tile([128, 4, VE + D], f32)
            nc.sync.dma_start(wm_sb[:], wmT_in.rearrange("(t p) k -> p t k", p=128))
            wp_sb = const.tile([128, 4, OUT], f32)
            nc.sync.dma_start(wp_sb[:], wp_in.rearrange("(t p) o -> p t o", p=128))
            bm_sb = const.tile([128, 4], f32)
            nc.sync.dma_start(bm_sb[:], bm_in.rearrange("(t p) -> p t", p=128))
            bp_sb = const.tile([1, OUT], f32)
            nc.sync.dma_start(bp_sb[:], bp_in[:, :])

            # ---- fold W_eff = W_mora @ W_post, b_eff = b_mora @ W_post + b_post ----
            weffA = const.tile([KA, OUT], f32)   # rows 0..63: emb part, row 64: b_eff
            weffB0 = const.tile([128, OUT], f32)
            weffB1 = const.tile([128, OUT], f32)
            for g, (k0, k1, dstap) in enumerate((
                    (0, VE, weffA[0:VE, :]),
                    (VE, VE + 128, weffB0[:]),
                    (VE + 128, VE + 256, weffB1[:]))):
                pw = psX.tile([k1 - k0, OUT], f32, tag="psX")
                for t in range(4):
                    nc.tensor.matmul(pw[:], lhsT=wm_sb[:, t, k0:k1], rhs=wp_sb[:, t, :],
                                     start=(t == 0), stop=(t == 3))
                nc.vector.tensor_copy(dstap, pw[:])
            pbe = psX.tile([1, OUT], f32, tag="psX")
            for t in range(4):
                nc.tensor.matmul(pbe[:], lhsT=bm_sb[:, t:t + 1], rhs=wp_sb[:, t, :],
                                 start=(t == 0), stop=(t == 3))
            nc.vector.tensor_tensor(weffA[VE:KA, :], pbe[:], bp_sb[:], op=ALU.add)

            # ---- per-utterance pipeline ----
            for u in range(U):
                # vowel one-hot -> emb^T [VE, M] -> concatA [KA, M]
                vow_i = sb.tile([1, M], i32, tag="vowi")
                nc.sync.dma_start(vow_i[:], vow_in[u:u + 1, :])
                vow_f = sb.tile([1, M], f32, tag="vowf")
                nc.vector.tensor_copy(vow_f[:], vow_i[:])
                pvb = psX.tile([V, M], f32, tag="psX")
                nc.tensor.matmul(pvb[:], lhsT=ones_row[:, 0:V], rhs=vow_f[:],
                                 start=True, stop=True)
                oh_v = sb.tile([V, M], f32, tag="ohv")
                nc.vector.tensor_scalar(oh_v[:], pvb[:], iotac_f[0:V, :], 0.0,
                                        ALU.subtract, ALU.is_equal)
                pemb = psX.tile([VE, M], f32, tag="psX")
                nc.tensor.matmul(pemb[:], lhsT=emb_sb[:], rhs=oh_v[:],
                                 start=True, stop=True)
                concatA = sb.tile([KA, M], f32, tag="concatA")
                nc.vector.tensor_copy(concatA[0:VE, :], pemb[:])
                nc.vector.memset(concatA[VE:KA, :], 1.0)

                # mora_index, laid out [p, s*2+i] = idx[256 s + 2 p + i] (host-prepped)
                idx_i = sb.tile([128, SC * 2], i32, tag="idxi")
                nc.sync.dma_start(idx_i[:], morat_in[u, :, :])
                idx_f = sb.tile([128, SC * 2], f32, tag="idxf")
                nc.vector.tensor_copy(idx_f[:], idx_i[:])

                # segment sums^T and counts
                ps0 = psA.tile([128, M], f32, tag="psA")
                ps1 = psB.tile([128, M], f32, tag="psB")
                pcnt = psC.tile([1, M], f32, tag="psC")
                for q in range(NQ):
                    ft = featp.tile([128, SC // NQ, 2, D], f32, tag="feat")
                    nc.sync.dma_start(
                        ft[:],
                        feat_in[u, q * (F // NQ):(q + 1) * (F // NQ), :]
                        .rearrange("(s p i) d -> p s i d", p=128, i=2))
                    for sl in range(SC // NQ):
                        s = q * (SC // NQ) + sl
                        for i in range(2):
                            first = (s == 0 and i == 0)
                            last = (s == SC - 1 and i == 1)
                            col = idx_f[:, 2 * s + i:2 * s + i + 1]
                            if first or last:
                                oh = ohp.tile([128, M], f32, tag="ohfull")
                                nc.vector.tensor_scalar(
                                    oh[:], iota_f[:], col, 0.0,
                                    ALU.subtract, ALU.is_equal)
                                o0, o1, oc = ps0[:], ps1[:], pcnt[:]
                            else:
                                st = starts[s]
                                oh = ohp.tile([128, win_w], f32, tag="ohwin")
                                nc.vector.tensor_scalar(
                                    oh[:], iota_f[:, 0:win_w], col, float(-st),
                                    ALU.subtract, ALU.is_equal)
                                o0 = ps0[:, st:st + win_w]
                                o1 = ps1[:, st:st + win_w]
                                oc = pcnt[:, st:st + win_w]
                            nc.tensor.matmul(o0, lhsT=ft[:, sl, i, 0:128], rhs=oh[:],
                                             start=first, stop=last,
                                             skip_group_check=not (first or last))
                            nc.tensor.matmul(o1, lhsT=ft[:, sl, i, 128:256], rhs=oh[:],
                                             start=first, stop=last,
                                             skip_group_check=not (first or last))
                            nc.tensor.matmul(oc, lhsT=ones_col[:], rhs=oh[:],
                                             start=first, stop=last,
                                             skip_group_check=not (first or last))

                # inv counts, broadcast across partitions, scale sums^T
                inv_sb = sb.tile([1, M], f32, tag="inv")
                nc.vector.tensor_scalar(inv_sb[:], pcnt[:], 1.0, None, ALU.max)
                nc.vector.reciprocal(inv_sb[:], inv_sb[:])
                pib = psX.tile([128, M], f32, tag="psX")
                nc.tensor.matmul(pib[:], lhsT=ones_row[:], rhs=inv_sb[:],
                                 start=True, stop=True)
                ib_sb = sb.tile([128, M], f32, tag="ib")
                nc.vector.tensor_copy(ib_sb[:], pib[:])
                b0 = sb.tile([128, M], f32, tag="b0")
                nc.vector.tensor_tensor(b0[:], ps0[:], ib_sb[:], op=ALU.mult)
                b1 = sb.tile([128, M], f32, tag="b1")
                nc.vector.tensor_tensor(b1[:], ps1[:], ib_sb[:], op=ALU.mult)

                # out^T [8, M] = W_effA.T @ concatA + W_effB0.T @ b0 + W_effB1.T @ b1
                po = psX.tile([OUT, M], f32, tag="psX")
                nc.tensor.matmul(po[:], lhsT=weffA[:], rhs=concatA[:],
                                 start=True, stop=False)
                nc.tensor.matmul(po[:], lhsT=weffB0[:], rhs=b0[:],
                                 start=False, stop=False)
                nc.tensor.matmul(po[:], lhsT=weffB1[:], rhs=b1[:],
                                 start=False, stop=True)
                out_sb = sb.tile([OUT, M], f32, tag="outsb")
                nc.vector.tensor_copy(out_sb[:], po[:])
                nc.sync.dma_start(out_dram[u, :, :], out_sb[:])

    nc.compile()
    return nc


def kernel(**inputs):
    global LAST_EXEC_NS
    bass, tile, bacc, mybir = _import_bass()
    from concourse.bass_utils import run_bass_kernel_spmd

    features = np.asarray(inputs["features"], dtype=np.float32)
    vowels = np.asarray(inputs["vowels"]).astype(np.int32)
    mora = np.asarray(inputs["mora_index"]).astype(np.int32)
    emb = np.asarray(inputs["emb_table"], dtype=np.float32)
    W_mora = np.asarray(inputs["W_mora"], dtype=np.float32)
    b_mora = np.asarray(inputs["b_mora"], dtype=np.float32)
    W_post = np.asarray(inputs["W_post"], dtype=np.float32)
    b_post = np.asarray(inputs["b_post"], dtype=np.float32)

    win_w, starts = _window_schedule(mora)
    key = (win_w, starts)
    if key not in _cache:
        _cache[key] = _build_nc(win_w, starts)
    nc = _cache[key]

    # host layout prep
    wmT = np.ascontiguousarray(W_mora.T)                       # [512, 320]
    bp = np.ascontiguousarray(b_post.reshape(1, OUT))
    # morat[u, p, 2 s + i] = mora[u, 256 s + 2 p + i]
    morat = np.ascontiguousarray(
        mora.reshape(B, SC, 128, 2).transpose(0, 2, 1, 3).reshape(B, 128, SC * 2))

    in_maps = []
    for k in range(N_CORES):
        sl = slice(U * k, U * (k + 1))
        in_maps.append({
            "features": np.ascontiguousarray(features[sl]),
            "morat": np.ascontiguousarray(morat[sl]),
            "vowels": np.ascontiguousarray(vowels[sl]),
            "emb": emb,
            "W_moraT": wmT,
            "W_post": np.ascontiguousarray(W_post),
            "b_mora": np.ascontiguousarray(b_mora),
            "b_post": bp,
        })

    if _TRACE:
        try:
            from antenv import axon_hooks
            if axon_hooks.get_axon_ntff_profile_hook() is None:
                from trn_agent_boot.trn_boot import _ntff_profile_via_ctypes
                hook = _ntff_profile_via_ctypes("/opt/axon/libaxon_pjrt.so")
                if hook is not None:
                    axon_hooks.set_axon_ntff_profile_hook(hook)
        except Exception:
            pass

    res = run_bass_kernel_spmd(nc, in_maps, list(range(N_CORES)), trace=_TRACE)
    LAST_EXEC_NS = res.exec_time_ns

    outT = np.concatenate([res.results[k]["out"] for k in range(N_CORES)], axis=0)
    out = outT.transpose(0, 2, 1).reshape(B, M, 2, 4)
    return np.ascontiguousarray(out.astype(np.float32))


# revision 12
# speedup vs baseline: 1.0099x; 1.0099x over previous
"""Trainium2 Bass kernel for nn_Predictor (segment-mean + embedding + fused linears).

Model (reference):
    mora_feat = segment_mean(features, mora_index)        # [B, M, D], sorted contiguous segments
    mv        = emb_table[vowels]                          # [B, M, VE]
    mh        = concat([mv, mora_feat]) @ W_mora + b_mora  # [B, M, H]
    (fh = features @ W_frame + b_frame is computed then deleted -> dead code, skipped)
    out       = mh @ W_post + b_post                       # [B, M, 8] -> [B, M, 2, 4]

Since there is no nonlinearity between the two linears they fold:
    out = concat([mv, mora_feat]) @ (W_mora @ W_post) + (b_mora @ W_post + b_post)
W_eff = W_mora @ W_post and b_eff are computed once on device.

Strategy (8 cores, data-parallel over batch: 2 utterances/core):
  - segment sums computed on TensorE as feat_chunk.T @ onehot(mora_index) with the
    output laid out transposed: sums^T [D, M].  Because mora_index is sorted, each
    256-frame superchunk only touches a narrow static window of mora columns; the
    window schedule is derived from the actual input at trace time (falls back to
    full width if the data violates it), so the single SPMD trace is correct for
    the given inputs by construction.
  - counts via a ones-column matmul into a [1, M] psum row; mean scaling is folded
    in as a per-column multiply (inv counts broadcast via a K=1 matmul).
  - vowel embedding gather as emb_table.T @ onehot(vowels) (V=50 is tiny).
  - final: out^T [8, M] = W_eff_aug.T @ concat_aug^T with bias via a ones row.
"""

import os
import sys

import numpy as np

B, F, M, D = 16, 4096, 512, 256
VE, H, V, OUT = 64, 512, 50, 8
N_CORES = 8
U = B // N_CORES          # utterances per core
SC = 16                   # superchunks per utterance (256 frames each)
FPS = F // SC             # frames per superchunk = 256
NQ = 4                    # feature DMAs per utterance (4 superchunks = 1 MiB each)

_TRACE = bool(os.environ.get("KERNEL_TRACE"))
LAST_EXEC_NS = None
LAST_RESULT = None

_cache = {}


def _import_bass():
    for p in ("/opt/trn_rl_repo",):
        if p not in sys.path:
            sys.path.insert(0, p)
    import concourse.bass as bass
    import concourse.tile as tile
    from concourse import bacc, mybir
    return bass, tile, bacc, mybir


def _window_schedule(mora):
    """Static per-superchunk mora windows covering every utterance's data."""
    lo = np.full(SC, 0, np.int64)
    hi = np.full(SC, M - 1, np.int64)
    for s in range(SC):
        seg = mora[:, s * FPS:(s + 1) * FPS]
        lo[s] = int(seg.min())
        hi[s] = int(seg.max())
    w = int((hi - lo + 1).max())
    w = min(M, max(32, ((w + 15) // 16) * 16))
    starts = np.minimum(lo, M - w).astype(np.int64)
    # sanity: windows must cover the data (always true by construction)
    assert all(lo[s] >= starts[s] and hi[s] < starts[s] + w for s in range(SC))
    return int(w), tuple(int(x) for x in starts)


def _build_nc(win_w, starts):
    bass, tile, bacc, mybir = _import_bass()
    from contextlib import ExitStack
    f32 = mybir.dt.float32
    i32 = mybir.dt.int32
    ALU = mybir.AluOpType

    nc = bacc.Bacc()
    feat_in = nc.declare_dram_parameter("features", [U, F, D], f32, isOutput=False)
    morat_in = nc.declare_dram_parameter("morat", [U, 128, SC * 2], i32, isOutput=False)
    vow_in = nc.declare_dram_parameter("vowels", [U, M], i32, isOutput=False)
    emb_in = nc.declare_dram_parameter("emb", [V, VE], f32, isOutput=False)
    wmT_in = nc.declare_dram_parameter("W_moraT", [H, VE + D], f32, isOutput=False)
    wp_in = nc.declare_dram_parameter("W_post", [H, OUT], f32, isOutput=False)
    bm_in = nc.declare_dram_parameter("b_mora", [H], f32, isOutput=False)
    bp_in = nc.declare_dram_parameter("b_post", [1, OUT], f32, isOutput=False)
    out_dram = nc.declare_dram_parameter("out", [U, OUT, M], f32, isOutput=True)

    KA = VE + 1  # emb rows + ones row (bias)

    with tile.TileContext(nc) as tc:
        with ExitStack() as ctx:
            const = ctx.enter_context(tc.tile_pool(name="const", bufs=1))
            sb = ctx.enter_context(tc.tile_pool(name="sb", bufs=2))
            featp = ctx.enter_context(tc.tile_pool(name="featp", bufs=4))
            ohp = ctx.enter_context(tc.tile_pool(name="ohp", bufs=4))
            psA = ctx.enter_context(tc.tile_pool(name="psA", bufs=2, space="PSUM"))
            psB = ctx.enter_context(tc.tile_pool(name="psB", bufs=2, space="PSUM"))
            psC = ctx.enter_context(tc.tile_pool(name="psC", bufs=2, space="PSUM"))
            psX = ctx.enter_context(tc.tile_pool(name="psX", bufs=2, space="PSUM"))

            # ---- constants ----
            iota_i = const.tile([128, M], i32)
            nc.gpsimd.iota(iota_i[:], pattern=[[1, M]], base=0, channel_multiplier=0)
            iota_f = const.tile([128, M], f32)
            nc.vector.tensor_copy(iota_f[:], iota_i[:])
            iotac_i = const.tile([128, 1], i32)
            nc.gpsimd.iota(iotac_i[:], pattern=[[1, 1]], base=0, channel_multiplier=1)
            iotac_f = const.tile([128, 1], f32)
            nc.vector.tensor_copy(iotac_f[:], iotac_i[:])
            ones_col = const.tile([128, 1], f32)
            nc.vector.memset(ones_col[:], 1.0)
            ones_row = const.tile([1, 128], f32)
            nc.vector.memset(ones_row[:], 1.0)

            emb_sb = const.tile([V, VE], f32)
            nc.sync.dma_start(emb_sb[:], emb_in[:, :])
            wm_sb = const.tile([128, 4, VE + D], f32)
            nc.sync.dma_start(wm_sb[:], wmT_in.rearrange("(t p) k -> p t k", p=128))
            wp_sb = const.tile([128, 4, OUT], f32)
            nc.sync.dma_start(wp_sb[:], wp_in.rearrange("(t p) o -> p t o", p=128))
            bm_sb = const.tile([128, 4], f32)
            nc.sync.dma_start(bm_sb[:], bm_in.rearrange("(t p) -> p t", p=128))
            bp_sb = const.tile([1, OUT], f32)
            nc.sync.dma_start(bp_sb[:], bp_in[:, :])

            # ---- fold W_eff = W_mora @ W_post, b_eff = b_mora @ W_post + b_post ----
            weffA = const.tile([KA, OUT], f32)   # rows 0..63: emb part, row 64: b_eff
            weffB0 = const.tile([128, OUT], f32)
            weffB1 = const.tile([128, OUT], f32)
            for g, (k0, k1, dstap) in enumerate((
                    (0, VE, weffA[0:VE, :]),
                    (VE, VE + 128, weffB0[:]),
                    (VE + 128, VE + 256, weffB1[:]))):
                pw = psX.tile([k1 - k0, OUT], f32, tag="psX")
                for t in range(4):
                    nc.tensor.matmul(pw[:], lhsT=wm_sb[:, t, k0:k1], rhs=wp_sb[:, t, :],
                                     start=(t == 0), stop=(t == 3))
                nc.vector.tensor_copy(dstap, pw[:])
            pbe = psX.tile([1, OUT], f32, tag="psX")
            for t in range(4):
                nc.tensor.matmul(pbe[:], lhsT=bm_sb[:, t:t + 1], rhs=wp_sb[:, t, :],
                                 start=(t == 0), stop=(t == 3))
            nc.vector.tensor_tensor(weffA[VE:KA, :], pbe[:], bp_sb[:], op=ALU.add)

            # ---- per-utterance pipeline ----
            for u in range(U):
                # vowel one-hot -> emb^T [VE, M] -> concatA [KA, M]
                vow_i = sb.tile([1, M], i32, tag="vowi")
                nc.sync.dma_start(vow_i[:], vow_in[u:u + 1, :])
                vow_f = sb.tile([1, M], f32, tag="vowf")
                nc.vector.tensor_copy(vow_f[:], vow_i[:])
                pvb = psX.tile([V, M], f32, tag="psX")
                nc.tensor.matmul(pvb[:], lhsT=ones_row[:, 0:V], rhs=vow_f[:],
                                 start=True, stop=True)
                oh_v = sb.tile([V, M], f32, tag="ohv")
                nc.vector.tensor_scalar(oh_v[:], pvb[:], iotac_f[0:V, :], 0.0,
                                        ALU.subtract, ALU.is_equal)
                pemb = psX.tile([VE, M], f32, tag="psX")
                nc.tensor.matmul(pemb[:], lhsT=emb_sb[:], rhs=oh_v[:],
                                 start=True, stop=True)
                concatA = sb.tile([KA, M], f32, tag="concatA")
                nc.vector.tensor_copy(concatA[0:VE, :], pemb[:])
                nc.vector.memset(concatA[VE:KA, :], 1.0)

                # mora_index, laid out [p, s*2+i] = idx[256 s + 2 p + i] (host-prepped)
                idx_i = sb.tile([128, SC * 2], i32, tag="idxi")
                nc.sync.dma_start(idx_i[:], morat_in[u, :, :])
                idx_f = sb.tile([128, SC * 2], f32, tag="idxf")
                nc.vector.tensor_copy(idx_f[:], idx_i[:])

                # segment sums^T and counts
                ps0 = psA.tile([128, M], f32, tag="psA")
                ps1 = psB.tile([128, M], f32, tag="psB")
                pcnt = psC.tile([1, M], f32, tag="psC")
                for q in range(NQ):
                    ft = featp.tile([128, SC // NQ, 2, D], f32, tag="feat")
                    nc.sync.dma_start(
                        ft[:],
                        feat_in[u, q * (F // NQ):(q + 1) * (F // NQ), :]
                        .rearrange("(s p i) d -> p s i d", p=128, i=2))
                    for sl in range(SC // NQ):
                        s = q * (SC // NQ) + sl
                        for i in range(2):
                            first = (s == 0 and i == 0)
                            last = (s == SC - 1 and i == 1)
                            col = idx_f[:, 2 * s + i:2 * s + i + 1]
                            if first or last:
                                oh = ohp.tile([128, M], f32, tag="ohfull")
                                nc.vector.tensor_scalar(
                                    oh[:], iota_f[:], col, 0.0,
                                    ALU.subtract, ALU.is_equal)
                                o0, o1, oc = ps0[:], ps1[:], pcnt[:]
                            else:
                                st = starts[s]
                                oh = ohp.tile([128, win_w], f32, tag="ohwin")
                                nc.vector.tensor_scalar(
                                    oh[:], iota_f[:, 0:win_w], col, float(-st),
                                    ALU.subtract, ALU.is_equal)
                                o0 = ps0[:, st:st + win_w]
                                o1 = ps1[:, st:st + win_w]
                                oc = pcnt[:, st:st + win_w]
                            nc.tensor.matmul(o0, lhsT=ft[:, sl, i, 0:128], rhs=oh[:],
                                             start=first, stop=last,
                                             skip_group_check=not (first or last))
                            nc.tensor.matmul(o1, lhsT=ft[:, sl, i, 128:256], rhs=oh[:],
                                             start=first, stop=last,
                                             skip_group_check=not (first or last))
                            nc.tensor.matmul(oc, lhsT=ones_col[:], rhs=oh[:],
                                             start=first, stop=last,
                                             skip_group_check=not (first or last))

                # inv counts, broadcast across partitions, scale sums^T
                inv_sb = sb.tile([1, M], f32, tag="inv")
                nc.vector.tensor_scalar(inv_sb[:], pcnt[:], 1.0, None, ALU.max)
                nc.vector.reciprocal(inv_sb[:], inv_sb[:])
                pib = psX.tile([128, M], f32, tag="psX")
                nc.tensor.matmul(pib[:], lhsT=ones_row[:], rhs=inv_sb[:],
                                 start=True, stop=True)
                ib_sb = sb.tile([128, M], f32, tag="ib")
                nc.vector.tensor_copy(ib_sb[:], pib[:])
                b0 = sb.tile([128, M], f32, tag="b0")
                nc.vector.tensor_tensor(b0[:], ps0[:], ib_sb[:], op=ALU.mult)
                b1 = sb.tile([128, M], f32, tag="b1")
                nc.vector.tensor_tensor(b1[:], ps1[:], ib_sb[:], op=ALU.mult)

                # out^T [8, M] = W_effA.T @ concatA + W_effB0.T @ b0 + W_effB1.T @ b1
                po = psX.tile([OUT, M], f32, tag="psX")
                nc.tensor.matmul(po[:], lhsT=weffA[:], rhs=concatA[:],
                                 start=True, stop=False)
                nc.tensor.matmul(po[:], lhsT=weffB0[:], rhs=b0[:],
                                 start=False, stop=False)
                nc.tensor.matmul(po[:], lhsT=weffB1[:], rhs=b1[:],
                                 start=False, stop=True)
                out_sb = sb.tile([OUT, M], f32, tag="outsb")
                nc.vector.tensor_copy(out_sb[:], po[:])
                nc.sync.dma_start(out_dram[u, :, :], out_sb[:])

    nc.compile()
    return nc


def kernel(**inputs):
    global LAST_EXEC_NS
    bass, tile, bacc, mybir = _import_bass()
    from concourse.bass_utils import run_bass_kernel_spmd

    features = np.asarray(inputs["features"], dtype=np.float32)
    vowels = np.asarray(inputs["vowels"]).astype(np.int32)
    mora = np.asarray(inputs["mora_index"]).astype(np.int32)
    emb = np.asarray(inputs["emb_table"], dtype=np.float32)
    W_mora = np.asarray(inputs["W_mora"], dtype=np.float32)
    b_mora = np.asarray(inputs["b_mora"], dtype=np.float32)
    W_post = np.asarray(inputs["W_post"], dtype=np.float32)
    b_post = np.asarray(inputs["b_post"], dtype=np.float32)

    win_w, starts = _window_schedule(mora)
    key = (win_w, starts)
    if key not in _cache:
        _cache[key] = _build_nc(win_w, starts)
    nc = _cache[key]

    # host layout prep
    wmT = np.ascontiguousarray(W_mora.T)                       # [512, 320]
    bp = np.ascontiguousarray(b_post.reshape(1, OUT))
    # morat[u, p, 2 s + i] = mora[u, 256 s + 2 p + i]
    morat = np.ascontiguousarray(
        mora.reshape(B, SC, 128, 2).transpose(0, 2, 1, 3).reshape(B, 128, SC * 2))

    in_maps = []
    for k in range(N_CORES):
        sl = slice(U * k, U * (k + 1))
        in_maps.append({
            "features": np.ascontiguousarray(features[sl]),
            "morat": np.ascontiguousarray(morat[sl]),
            "vowels": np.ascontiguousarray(vowels[sl]),
            "emb": emb,
            "W_moraT": wmT,
            "W_post": np.ascontiguousarray(W_post),
            "b_mora": np.ascontiguousarray(b_mora),
            "b_post": bp,
        })

    if _TRACE:
        try:
            from antenv import axon_hooks
            if axon_hooks.get_axon_ntff_profile_hook() is None:
                from trn_agent_boot.trn_boot import _ntff_profile_via_ctypes
                hook = _ntff_profile_via_ctypes("/opt/axon/libaxon_pjrt.so")
                if hook is not None:
                    axon_hooks.set_axon_ntff_profile_hook(hook)
        except Exception:
            pass

    res = run_bass_kernel_spmd(nc, in_maps, list(range(N_CORES)), trace=_TRACE)
    LAST_EXEC_NS = res.exec_time_ns
    global LAST_RESULT
    LAST_RESULT = res

    outT = np.concatenate([res.results[k]["out"] for k in range(N_CORES)], axis=0)
    out = outT.transpose(0, 2, 1).reshape(B, M, 2, 4)
    return np.ascontiguousarray(out.astype(np.float32))


# revision 14
# speedup vs baseline: 1.3484x; 1.3351x over previous
"""Trainium2 Bass kernel for nn_Predictor (segment-mean + embedding + fused linears).

Model (reference):
    mora_feat = segment_mean(features, mora_index)        # [B, M, D], sorted contiguous segments
    mv        = emb_table[vowels]                          # [B, M, VE]
    mh        = concat([mv, mora_feat]) @ W_mora + b_mora  # [B, M, H]
    (fh = features @ W_frame + b_frame is computed then deleted -> dead code, skipped)
    out       = mh @ W_post + b_post                       # [B, M, 8] -> [B, M, 2, 4]

Since there is no nonlinearity between the two linears they fold:
    out = concat([mv, mora_feat]) @ (W_mora @ W_post) + (b_mora @ W_post + b_post)
W_eff = W_mora @ W_post and b_eff are computed once on device.

Strategy (8 cores, data-parallel over batch: 2 utterances/core):
  - segment sums computed on TensorE as feat_chunk.T @ onehot(mora_index) with the
    output laid out transposed: sums^T [D, M].  Because mora_index is sorted, each
    256-frame superchunk only touches a narrow static window of mora columns; the
    window schedule is derived from the actual input at trace time (falls back to
    full width if the data violates it), so the single SPMD trace is correct for
    the given inputs by construction.
  - counts via a ones-column matmul into a [1, M] psum row; mean scaling is folded
    in as a per-column multiply (inv counts broadcast via a K=1 matmul).
  - vowel embedding gather as emb_table.T @ onehot(vowels) (V=50 is tiny).
  - final: out^T [8, M] = W_eff_aug.T @ concat_aug^T with bias via a ones row.
"""

import os
import sys

import numpy as np

B, F, M, D = 16, 4096, 512, 256
VE, H, V, OUT = 64, 512, 50, 8
N_CORES = 8
U = B // N_CORES          # utterances per core
SC = 16                   # superchunks per utterance (256 frames each)
FPS = F // SC             # frames per superchunk = 256
NQ = 4                    # feature DMAs per utterance (4 superchunks = 1 MiB each)

_TRACE = bool(os.environ.get("KERNEL_TRACE"))
LAST_EXEC_NS = None
LAST_RESULT = None

_cache = {}


def _import_bass():
    for p in ("/opt/trn_rl_repo",):
        if p not in sys.path:
            sys.path.insert(0, p)
    import concourse.bass as bass
    import concourse.tile as tile
    from concourse import bacc, mybir
    return bass, tile, bacc, mybir


def _window_schedule(mora):
    """Static per-superchunk mora windows covering every utterance's data."""
    lo = np.full(SC, 0, np.int64)
    hi = np.full(SC, M - 1, np.int64)
    for s in range(SC):
        seg = mora[:, s * FPS:(s + 1) * FPS]
        lo[s] = int(seg.min())
        hi[s] = int(seg.max())
    w = int((hi - lo + 1).max())
    w = min(M, max(32, ((w + 15) // 16) * 16))
    starts = np.minimum(lo, M - w).astype(np.int64)
    # sanity: windows must cover the data (always true by construction)
    assert all(lo[s] >= starts[s] and hi[s] < starts[s] + w for s in range(SC))
    return int(w), tuple(int(x) for x in starts)


def _build_nc(win_w, starts):
    bass, tile, bacc, mybir = _import_bass()
    from contextlib import ExitStack
    f32 = mybir.dt.float32
    bf16 = mybir.dt.bfloat16
    i32 = mybir.dt.int32
    ALU = mybir.AluOpType

    nc = bacc.Bacc()
    feat_in = nc.declare_dram_parameter("features", [U, F, D], f32, isOutput=False)
    morat_in = nc.declare_dram_parameter("morat", [U, 128, SC * 2], i32, isOutput=False)
    vow_in = nc.declare_dram_parameter("vowels", [U, M], i32, isOutput=False)
    emb_in = nc.declare_dram_parameter("emb", [V, VE], f32, isOutput=False)
    wmT_in = nc.declare_dram_parameter("W_moraT", [H, VE + D], f32, isOutput=False)
    wp_in = nc.declare_dram_parameter("W_post", [H, OUT], f32, isOutput=False)
    bm_in = nc.declare_dram_parameter("b_mora", [H], f32, isOutput=False)
    bp_in = nc.declare_dram_parameter("b_post", [1, OUT], f32, isOutput=False)
    out_dram = nc.declare_dram_parameter("out", [U, OUT, M], f32, isOutput=True)

    KA = VE + 1  # emb rows + ones row (bias)

    with tile.TileContext(nc) as tc:
        with ExitStack() as ctx:
            const = ctx.enter_context(tc.tile_pool(name="const", bufs=1))
            sb = ctx.enter_context(tc.tile_pool(name="sb", bufs=2))
            featp = ctx.enter_context(tc.tile_pool(name="featp", bufs=6))
            ohp = ctx.enter_context(tc.tile_pool(name="ohp", bufs=4))
            psA = ctx.enter_context(tc.tile_pool(name="psA", bufs=2, space="PSUM"))
            psB = ctx.enter_context(tc.tile_pool(name="psB", bufs=2, space="PSUM"))
            psC = ctx.enter_context(tc.tile_pool(name="psC", bufs=2, space="PSUM"))
            psX = ctx.enter_context(tc.tile_pool(name="psX", bufs=2, space="PSUM"))

            # ---- constants ----
            iota_i = const.tile([128, M], i32)
            nc.gpsimd.iota(iota_i[:], pattern=[[1, M]], base=0, channel_multiplier=0)
            iota_f = const.tile([128, M], f32)
            nc.vector.tensor_copy(iota_f[:], iota_i[:])
            iotac_i = const.tile([128, 1], i32)
            nc.gpsimd.iota(iotac_i[:], pattern=[[1, 1]], base=0, channel_multiplier=1)
            iotac_f = const.tile([128, 1], f32)
            nc.vector.tensor_copy(iotac_f[:], iotac_i[:])
            ones_col = const.tile([128, 1], bf16)
            nc.vector.memset(ones_col[:], 1.0)
            ones_row = const.tile([1, 128], f32)
            nc.vector.memset(ones_row[:], 1.0)

            emb_sb = const.tile([V, VE], f32)
            nc.sync.dma_start(emb_sb[:], emb_in[:, :])
            wm_sb = const.tile([128, 4, VE + D], f32)
            nc.sync.dma_start(wm_sb[:], wmT_in.rearrange("(t p) k -> p t k", p=128))
            wp_sb = const.tile([128, 4, OUT], f32)
            nc.sync.dma_start(wp_sb[:], wp_in.rearrange("(t p) o -> p t o", p=128))
            bm_sb = const.tile([128, 4], f32)
            nc.sync.dma_start(bm_sb[:], bm_in.rearrange("(t p) -> p t", p=128))
            bp_sb = const.tile([1, OUT], f32)
            nc.sync.dma_start(bp_sb[:], bp_in[:, :])

            # ---- fold W_eff = W_mora @ W_post, b_eff = b_mora @ W_post + b_post ----
            weffA = const.tile([KA, OUT], f32)   # rows 0..63: emb part, row 64: b_eff
            weffB0 = const.tile([128, OUT], f32)
            weffB1 = const.tile([128, OUT], f32)
            for g, (k0, k1, dstap) in enumerate((
                    (0, VE, weffA[0:VE, :]),
                    (VE, VE + 128, weffB0[:]),
                    (VE + 128, VE + 256, weffB1[:]))):
                pw = psX.tile([k1 - k0, OUT], f32, tag="psX")
                for t in range(4):
                    nc.tensor.matmul(pw[:], lhsT=wm_sb[:, t, k0:k1], rhs=wp_sb[:, t, :],
                                     start=(t == 0), stop=(t == 3))
                nc.vector.tensor_copy(dstap, pw[:])
            pbe = psX.tile([1, OUT], f32, tag="psX")
            for t in range(4):
                nc.tensor.matmul(pbe[:], lhsT=bm_sb[:, t:t + 1], rhs=wp_sb[:, t, :],
                                 start=(t == 0), stop=(t == 3))
            nc.vector.tensor_tensor(weffA[VE:KA, :], pbe[:], bp_sb[:], op=ALU.add)

            # ---- per-utterance pipeline ----
            for u in range(U):
                # vowel one-hot -> emb^T [VE, M] -> concatA [KA, M]
                vow_i = sb.tile([1, M], i32, tag="vowi")
                nc.sync.dma_start(vow_i[:], vow_in[u:u + 1, :])
                vow_f = sb.tile([1, M], f32, tag="vowf")
                nc.vector.tensor_copy(vow_f[:], vow_i[:])
                pvb = psX.tile([V, M], f32, tag="psX")
                nc.tensor.matmul(pvb[:], lhsT=ones_row[:, 0:V], rhs=vow_f[:],
                                 start=True, stop=True)
                oh_v = sb.tile([V, M], f32, tag="ohv")
                nc.vector.tensor_scalar(oh_v[:], pvb[:], iotac_f[0:V, :], 0.0,
                                        ALU.subtract, ALU.is_equal)
                pemb = psX.tile([VE, M], f32, tag="psX")
                nc.tensor.matmul(pemb[:], lhsT=emb_sb[:], rhs=oh_v[:],
                                 start=True, stop=True)
                concatA = sb.tile([KA, M], f32, tag="concatA")
                nc.vector.tensor_copy(concatA[0:VE, :], pemb[:])
                nc.vector.memset(concatA[VE:KA, :], 1.0)

                # mora_index, laid out [p, s*2+i] = idx[256 s + 2 p + i] (host-prepped)
                idx_i = sb.tile([128, SC * 2], i32, tag="idxi")
                nc.sync.dma_start(idx_i[:], morat_in[u, :, :])
                idx_f = sb.tile([128, SC * 2], f32, tag="idxf")
                nc.vector.tensor_copy(idx_f[:], idx_i[:])

                # segment sums^T and counts
                ps0 = psA.tile([128, M], f32, tag="psA")
                ps1 = psB.tile([128, M], f32, tag="psB")
                pcnt = psC.tile([1, M], f32, tag="psC")
                for q in range(NQ):
                    ft = featp.tile([128, SC // NQ, 2, D], bf16, tag="feat")
                    nc.gpsimd.dma_start(
                        ft[:],
                        feat_in[u, q * (F // NQ):(q + 1) * (F // NQ), :]
                        .rearrange("(s p i) d -> p s i d", p=128, i=2))
                    for sl in range(SC // NQ):
                        s = q * (SC // NQ) + sl
                        for i in range(2):
                            first = (s == 0 and i == 0)
                            last = (s == SC - 1 and i == 1)
                            col = idx_f[:, 2 * s + i:2 * s + i + 1]
                            if first or last:
                                oh = ohp.tile([128, M], bf16, tag="ohfull")
                                nc.vector.tensor_scalar(
                                    oh[:], iota_f[:], col, 0.0,
                                    ALU.subtract, ALU.is_equal)
                                o0, o1, oc = ps0[:], ps1[:], pcnt[:]
                            else:
                                st = starts[s]
                                oh = ohp.tile([128, win_w], bf16, tag="ohwin")
                                nc.vector.tensor_scalar(
                                    oh[:], iota_f[:, 0:win_w], col, float(-st),
                                    ALU.subtract, ALU.is_equal)
                                o0 = ps0[:, st:st + win_w]
                                o1 = ps1[:, st:st + win_w]
                                oc = pcnt[:, st:st + win_w]
                            nc.tensor.matmul(o0, lhsT=ft[:, sl, i, 0:128], rhs=oh[:],
                                             start=first, stop=last,
                                             skip_group_check=not (first or last))
                            nc.tensor.matmul(o1, lhsT=ft[:, sl, i, 128:256], rhs=oh[:],
                                             start=first, stop=last,
                                             skip_group_check=not (first or last))
                            nc.tensor.matmul(oc, lhsT=ones_col[:], rhs=oh[:],
                                             start=first, stop=last,
                                             skip_group_check=not (first or last))

                # inv counts, broadcast across partitions, scale sums^T
                inv_sb = sb.tile([1, M], f32, tag="inv")
                nc.vector.tensor_scalar(inv_sb[:], pcnt[:], 1.0, None, ALU.max)
                nc.vector.reciprocal(inv_sb[:], inv_sb[:])
                pib = psX.tile([128, M], f32, tag="psX")
                nc.tensor.matmul(pib[:], lhsT=ones_row[:], rhs=inv_sb[:],
                                 start=True, stop=True)
                ib_sb = sb.tile([128, M], f32, tag="ib")
                nc.vector.tensor_copy(ib_sb[:], pib[:])
                b0 = sb.tile([128, M], f32, tag="b0")
                nc.vector.tensor_tensor(b0[:], ps0[:], ib_sb[:], op=ALU.mult)
                b1 = sb.tile([128, M], f32, tag="b1")
                nc.vector.tensor_tensor(b1[:], ps1[:], ib_sb[:], op=ALU.mult)

                # out^T [8, M] = W_effA.T @ concatA + W_effB0.T @ b0 + W_effB1.T @ b1
                po = psX.tile([OUT, M], f32, tag="psX")
                nc.tensor.matmul(po[:], lhsT=weffA[:], rhs=concatA[:],
                                 start=True, stop=False)
                nc.tensor.matmul(po[:], lhsT=weffB0[:], rhs=b0[:],
                                 start=False, stop=False)
                nc.tensor.matmul(po[:], lhsT=weffB1[:], rhs=b1[:],
                                 start=False, stop=True)
                out_sb = sb.tile([OUT, M], f32, tag="outsb")
                nc.vector.tensor_copy(out_sb[:], po[:])
                nc.sync.dma_start(out_dram[u, :, :], out_sb[:])

    nc.compile()
    return nc


def kernel(**inputs):
    global LAST_EXEC_NS
    bass, tile, bacc, mybir = _import_bass()
    from concourse.bass_utils import run_bass_kernel_spmd

    features = np.asarray(inputs["features"], dtype=np.float32)
    vowels = np.asarray(inputs["vowels"]).astype(np.int32)
    mora = np.asarray(inputs["mora_index"]).astype(np.int32)
    emb = np.asarray(inputs["emb_table"], dtype=np.float32)
    W_mora = np.asarray(inputs["W_mora"], dtype=np.float32)
    b_mora = np.asarray(inputs["b_mora"], dtype=np.float32)
    W_post = np.asarray(inputs["W_post"], dtype=np.float32)
    b_post = np.asarray(inputs["b_post"], dtype=np.float32)

    win_w, starts = _window_schedule(mora)
    key = (win_w, starts)
    if key not in _cache:
        _cache[key] = _build_nc(win_w, starts)
    nc = _cache[key]

    # host layout prep
    wmT = np.ascontiguousarray(W_mora.T)                       # [512, 320]
    bp = np.ascontiguousarray(b_post.reshape(1, OUT))
    # morat[u, p, 2 s + i] = mora[u, 256 s + 2 p + i]
    morat = np.ascontiguousarray(
        mora.reshape(B, SC, 128, 2).transpose(0, 2, 1, 3).reshape(B, 128, SC * 2))

    in_maps = []
    for k in range(N_CORES):
        sl = slice(U * k, U * (k + 1))
        in_maps.append({
            "features": np.ascontiguousarray(features[sl]),
            "morat": np.ascontiguousarray(morat[sl]),
            "vowels": np.ascontiguousarray(vowels[sl]),
            "emb": emb,
            "W_moraT": wmT,
            "W_post": np.ascontiguousarray(W_post),
            "b_mora": np.ascontiguousarray(b_mora),
            "b_post": bp,
        })

    if _TRACE:
        try:
            from antenv import axon_hooks
            if axon_hooks.get_axon_ntff_profile_hook() is None:
                from trn_agent_boot.trn_boot import _ntff_profile_via_ctypes
                hook = _ntff_profile_via_ctypes("/opt/axon/libaxon_pjrt.so")
                if hook is not None:
                    axon_hooks.set_axon_ntff_profile_hook(hook)
        except Exception:
            pass

    res = run_bass_kernel_spmd(nc, in_maps, list(range(N_CORES)), trace=_TRACE)
    LAST_EXEC_NS = res.exec_time_ns
    global LAST_RESULT
    LAST_RESULT = res

    outT = np.concatenate([res.results[k]["out"] for k in range(N_CORES)], axis=0)
    out = outT.transpose(0, 2, 1).reshape(B, M, 2, 4)
    return np.ascontiguousarray(out.astype(np.float32))
